# revision 1
# baseline (speedup 1.0000x reference)
"""Trainium2 Bass kernel for nn_DecodePredictions (YOLO-style decode, B=16).

Strategy: pure data-parallel over batch (2 images per core x 8 cores).
Host-side: concat the 3 prediction levels into a flat [N_anchor, 85] tensor
per image, pad 8400 -> 8448 anchors so everything divides evenly, and lay
anchors out partition-blocked so every DMA moves large contiguous
per-partition chunks. Score logits ship as fp8e4 (sigmoid rounding error
~1e-5 of absmax), box logits as fp32 planes; grid/stride constants are
precomputed host-side.

The whole output path is bf16: the gate is rel_err < 2e-2 against
absmax ~1958, and bf16 rounding of the box coords costs at most
ULP(2048)/2 = 4 absolute (~2e-3 relative), so halving the dominant
HBM write traffic is free accuracy-wise. The host upconverts to fp32
while scattering per-core results into the final array.

Device output layout is [anchor, lane, class] (lane-planes per anchor):
every SBUF write filling it is a contiguous run, which keeps DVE out of
its 0.3-elem/cycle scattered-write mode. The box planes (one value
repeated C times) are built in two stages: an int32 broadcast of
duplicated pairs into a 16-wide scratch, then a step-1 5x-replicating
copy into the output tile, so the expensive per-tile op runs in the
packed 2x/4x DVE modes. Host permutes [6,C]->[C,6] while upconverting.
"""

import ml_dtypes
import numpy as np

N_CORES = 8
B = 16
B_PER_CORE = B // N_CORES  # 2
C = 80
F = 85
N_REAL = 8400              # 80*80 + 40*40 + 20*20
N_PAD = 8448               # = 66 * 128
P = 128
KPP = B_PER_CORE * N_PAD // P  # 132 anchors per partition
GK = 11                    # anchors per partition per tile
NT = KPP // GK             # 12 tiles
NOB = 5                    # persistent output buffers
ICH = 33                   # anchors per input-chunk DMA (3 tiles)
NIC = KPP // ICH           # 4 input chunks

_CACHE: dict = {}


def _build_nc():
    import concourse.bacc as bacc
    import concourse.tile as tile
    from concourse import mybir
    from contextlib import ExitStack

    nc = bacc.Bacc("TRN2", target_bir_lowering=False, debug=False)
    pa01 = nc.dram_tensor("pa01", [P, KPP, 2], mybir.dt.float32, kind="ExternalInput")
    pa23 = nc.dram_tensor("pa23", [P, KPP, 2], mybir.dt.float32, kind="ExternalInput")
    auxS = nc.dram_tensor("auxS", [P, KPP, 2], mybir.dt.float32, kind="ExternalInput")
    auxB = nc.dram_tensor("auxB", [P, KPP, 2], mybir.dt.float32, kind="ExternalInput")
    predsB = nc.dram_tensor("predsB", [P, KPP, 81], mybir.dt.float8e4, kind="ExternalInput")
    clsc = nc.dram_tensor("clsc", [P, C], mybir.dt.bfloat16, kind="ExternalInput")
    out = nc.dram_tensor("out", [P, KPP, 6, C], mybir.dt.bfloat16, kind="ExternalOutput")

    fp32 = mybir.dt.float32
    bf16 = mybir.dt.bfloat16
    i32 = mybir.dt.int32
    AF = mybir.ActivationFunctionType

    with tile.TileContext(nc) as tc, ExitStack() as ctx:
        cpool = ctx.enter_context(tc.tile_pool(name="const", bufs=1))
        ipool = ctx.enter_context(tc.tile_pool(name="in", bufs=NIC))
        opool = ctx.enter_context(tc.tile_pool(name="out", bufs=1))
        tpool = ctx.enter_context(tc.tile_pool(name="tmp", bufs=NIC))

        # Consts on the Sync HWDGE ring (pa23 first: it gates the Exp that
        # gates everything); inputs + cls ride the second HWDGE ring (ACT)
        # so the two streams land concurrently.
        pa23_t = cpool.tile([P, KPP, 2], fp32, tag="pa23")
        nc.sync.dma_start(out=pa23_t[:], in_=pa23[:])
        pa01_t = cpool.tile([P, KPP, 2], fp32, tag="pa01")
        nc.sync.dma_start(out=pa01_t[:], in_=pa01[:])
        auxS_t = cpool.tile([P, KPP, 2], fp32, tag="auxS")
        nc.sync.dma_start(out=auxS_t[:], in_=auxS[:])
        auxB_t = cpool.tile([P, KPP, 2], fp32, tag="auxB")
        nc.sync.dma_start(out=auxB_t[:], in_=auxB[:])

        cls_t = cpool.tile([P, C], bf16, tag="cls")
        nc.scalar.dma_start(out=cls_t[:], in_=clsc[:])
        # Input chunks go on the SAME sync ring, behind the consts: the
        # SDMA engines round-robin rings at packet granularity, so putting
        # these on the other ring delays the consts (and the box decode that
        # gates everything) by ~5us. FIFO behind the consts they still land
        # before the sigmoids need them.
        in_tiles = []
        for ci in range(NIC):
            it = ipool.tile([P, ICH, 81], mybir.dt.float8e4, tag="pt", name=f"pt{ci}")
            nc.sync.dma_start(out=it[:], in_=predsB[:, ci * ICH : (ci + 1) * ICH, :])
            in_tiles.append(it)

        # Box decode in two halves (first 33 anchors, then the rest) so the
        # first output tile's chain is short. box_dup[p,k,l,d] duplicates
        # each corner value into an adjacent pair: (x1,x1),(y1,y1),(x2,x2),
        # (y2,y2) -- an int32 view then gives one register per repeated pair.
        wh_t = cpool.tile([P, KPP, 2], fp32, tag="wh")
        bb_t = cpool.tile([P, 2, KPP, 2], fp32, tag="bb")
        box_dup = cpool.tile([P, KPP, 4, 2], bf16, tag="boxd")
        nc.scalar.activation(wh_t[:], pa23_t[:], AF.Exp)
        nc.vector.tensor_mul(wh_t[:], wh_t[:], auxS_t[:])
        nc.vector.tensor_mul(bb_t[:, 0, :, :], pa01_t[:], auxS_t[:])
        nc.vector.tensor_add(bb_t[:, 0, :, :], bb_t[:, 0, :, :], auxB_t[:])
        nc.vector.tensor_add(bb_t[:, 1, :, :], bb_t[:, 0, :, :], wh_t[:])
        for jh in (0, 1):
            nc.vector.tensor_copy(
                box_dup[:, :, 2 * jh : 2 * jh + 2, :],
                bb_t[:, jh, :, :].unsqueeze(3).broadcast_to([P, KPP, 2, 2]),
            )

        # Persistent out buffers [anchor, lane, class]; constant class-id
        # plane (lane 4) written once per buffer on GpSimd (otherwise idle).
        ot_bufs = [
            opool.tile([P, GK, 6, C], bf16, tag=f"ot{j}", name=f"ot{j}")
            for j in range(NOB)
        ]
        for j in range(NOB):
            nc.gpsimd.tensor_copy(
                ot_bufs[j][:, :, 4, :],
                cls_t[:].unsqueeze(1).broadcast_to([P, GK, C]),
            )

        # Per chunk: sigmoid of the 80 class logits (contiguous), and
        # sigmoid of the objectness logit pre-broadcast across classes so
        # the per-tile score multiply is an all-step-1 TT (2x DVE mode).
        # Tile-0-sized sigmoid pair first (~1us each instead of 2.4) so the
        # first score TT unblocks as soon as the box path is ready.
        # Obj sigmoids broadcast only 16-wide: the score TT reads them via a
        # stride-0 middle dim (same AP shape as the stage-2 box copy), so
        # ACT's serial sigmoid ladder drops by ~2us per chunk.
        sc_t0 = tpool.tile([P, GK, 80], bf16, tag="sigct0")
        nc.scalar.activation(sc_t0[:], in_tiles[0][:, 0:GK, 1:81], AF.Sigmoid)
        so_t0 = tpool.tile([P, GK, 16], bf16, tag="sigot0")
        nc.scalar.activation(
            so_t0[:], in_tiles[0][:, 0:GK, 0:1].broadcast_to([P, GK, 16]), AF.Sigmoid
        )
        sig_cls, sig_obj = [], []
        for ci in range(NIC):
            sc = tpool.tile([P, ICH, 80], bf16, tag="sigc", name=f"sigc{ci}")
            nc.scalar.activation(sc[:], in_tiles[ci][:, :, 1:81], AF.Sigmoid)
            sig_cls.append(sc)
            so = tpool.tile([P, ICH, 16], bf16, tag="sigo", name=f"sigo{ci}")
            nc.scalar.activation(
                so[:], in_tiles[ci][:, :, 0:1].broadcast_to([P, ICH, 16]), AF.Sigmoid
            )
            sig_obj.append(so)

        for t in range(NT):
            sl = slice(t * GK, (t + 1) * GK)
            ci = t // 3
            ksl = slice((t % 3) * GK, (t % 3 + 1) * GK)
            ot = ot_bufs[t % NOB]

            # Stage 1: 8 copies of each duplicated int32 pair -> 16 repeats
            # of each bf16 corner value in scratch.
            rep = tpool.tile([P, GK, 4, 8], i32, tag="rep")
            nc.vector.tensor_copy(
                rep[:],
                box_dup[:, sl, :, :].bitcast(i32).broadcast_to([P, GK, 4, 8]),
            )
            # Stage 2: replicate the 16-wide runs 5x into the 80-wide box
            # lane planes -- src innermost is step-1 so DVE packs. The last
            # two tiles run on ACT, which is idle once the sigmoids drain,
            # to shorten DVE's tail.
            s2_out = ot[:, :, 0:4, :].rearrange("p k l (r c) -> p k l r c", r=5)
            s2_in = rep[:].bitcast(bf16).unsqueeze(3).broadcast_to([P, GK, 4, 5, 16])
            if t < NT - 2:
                nc.vector.tensor_copy(s2_out, s2_in)
            else:
                nc.scalar.copy(s2_out, s2_in)
            if t == 0:
                sc_in, so_in = sc_t0[:], so_t0[:]
            else:
                sc_in = sig_cls[ci][:, ksl, :]
                so_in = sig_obj[ci][:, ksl, :]
            nc.vector.tensor_mul(
                ot[:, :, 5, :].rearrange("p k (r c) -> p k r c", r=5),
                sc_in.rearrange("p k (r c) -> p k r c", r=5),
                so_in.unsqueeze(2).broadcast_to([P, GK, 5, 16]),
            )

            nc.sync.dma_start(out=out[:, sl, :, :], in_=ot[:])

    nc.compile()
    return nc


def _host_consts():
    # Per-anchor (stride, stride) and (gx*stride, gy*stride), padded to N_PAD.
    s = np.ones(N_PAD, np.float32)
    bx = np.zeros(N_PAD, np.float32)
    by = np.zeros(N_PAD, np.float32)
    off = 0
    for g, st in ((80, 8.0), (40, 16.0), (20, 32.0)):
        n = g * g
        i = np.arange(n)
        s[off : off + n] = st
        bx[off : off + n] = (i % g) * st
        by[off : off + n] = (i // g) * st
        off += n
    auxS = np.stack([s, s], axis=-1).astype(np.float32)
    auxB = np.stack([bx, by], axis=-1).astype(np.float32)
    auxS = np.concatenate([auxS] * B_PER_CORE, 0).reshape(P, KPP, 2)
    auxB = np.concatenate([auxB] * B_PER_CORE, 0).reshape(P, KPP, 2)
    cls = np.broadcast_to(
        np.arange(C, dtype=np.float32).astype(ml_dtypes.bfloat16), (P, C)
    ).copy()
    return np.ascontiguousarray(auxS), np.ascontiguousarray(auxB), cls


def _host_in_maps(pred0, pred1, pred2):
    auxS, auxB, cls = _CACHE["consts"]
    pred0 = np.asarray(pred0, np.float32).reshape(B, -1, F)
    pred1 = np.asarray(pred1, np.float32).reshape(B, -1, F)
    pred2 = np.asarray(pred2, np.float32).reshape(B, -1, F)
    in_maps = []
    for core in range(N_CORES):
        flat = np.zeros((B_PER_CORE * N_PAD, F), np.float32)
        for j in range(B_PER_CORE):
            b = core * B_PER_CORE + j
            flat[j * N_PAD : j * N_PAD + N_REAL] = np.concatenate(
                [pred0[b], pred1[b], pred2[b]], axis=0
            )
        in_maps.append(
            {
                "pa01": np.ascontiguousarray(flat[:, 0:2]).reshape(P, KPP, 2),
                "pa23": np.ascontiguousarray(flat[:, 2:4]).reshape(P, KPP, 2),
                "auxS": auxS,
                "auxB": auxB,
                "predsB": np.ascontiguousarray(flat[:, 4:85])
                .astype(ml_dtypes.float8_e4m3fn)
                .reshape(P, KPP, 81),
                "clsc": cls,
            }
        )
    return in_maps


def kernel(images, pred0, pred1, pred2):
    from concourse.bass_utils import run_bass_kernel_spmd

    if "nc" not in _CACHE:
        _CACHE["consts"] = _host_consts()
        _CACHE["nc"] = _build_nc()
    nc = _CACHE["nc"]

    in_maps = _host_in_maps(pred0, pred1, pred2)
    res = run_bass_kernel_spmd(nc, in_maps, list(range(N_CORES)))
    final = np.empty((B, N_REAL * C, 6), np.float32)
    for core, r in enumerate(res.results):
        # Device layout is [anchor, lane, C]; upconvert bf16 -> fp32 on the
        # contiguous array first (vectorized), then swap to [anchor, C, lane]
        # with an fp32 strided assign -- orders of magnitude faster than one
        # fused strided bf16 cast-assign.
        f32 = r["out"].reshape(B_PER_CORE, N_PAD, 6, C)[:, :N_REAL].astype(
            np.float32
        )
        final[core * B_PER_CORE : (core + 1) * B_PER_CORE].reshape(
            B_PER_CORE, N_REAL, C, 6
        )[:] = f32.transpose(0, 1, 3, 2)
    return final



# revision 2
# speedup vs baseline: 2.6310x; 2.6310x over previous
"""Trainium2 Bass kernel for nn_DecodePredictions (YOLO-style decode, B=16).

Pure data-parallel over batch (2 images per core x 8 cores).

The [B, N*C, 6] output is hugely redundant on the device side: per anchor,
the 4 box coords repeat across all 80 classes and lane 4 is the constant
class id. The device therefore emits only the per-anchor uniques --
4 box coords (bf16) and 80 class scores (bf16) -- ~1.5 MB/core instead of
the 16 MB/core the full layout costs; the host broadcasts them into the
full [B, N*C, 6] fp32 array while unsharding.

Score path: sigma(o)*sigma(c) = 0.25*(1+tanh(o/2))*(1+tanh(c/2)). Using
tanh instead of Sigmoid keeps every ACT function in the single
`exp_and_others` table set (exp is needed for box wh), so the kernel pays
one ACT_TABLE_LOAD instead of two, and the whole product folds into one
fused DVE scalar_tensor_tensor per chunk: (tanh_c + 1.0) * halfA, where
halfA = 0.5*sigma(obj) precomputed once per anchor.

Layouts are class-major [P, 81, KPP] (anchor innermost) so every ACT/DVE
op is step-1 contiguous (DVE 2x mode) and every DMA moves 128 contiguous
per-partition segments of 2-4 KB. Class-chunked to pipeline DMA-in ->
tanh -> STT -> DMA-out.
"""

import ml_dtypes
import numpy as np

N_CORES = 8
B = 16
B_PER_CORE = B // N_CORES  # 2
C = 80
F = 85
N_REAL = 8400              # 80*80 + 40*40 + 20*20
N_PAD = 8448               # = 66 * 128
P = 128
KPP = B_PER_CORE * N_PAD // P  # 132 anchors per partition
R = C + 1                  # obj row + 80 class rows
# tanh-row chunks [r0, r1); chunk 0 carries the obj row. Score rows are the
# same ranges shifted down by the obj row, so STT chunk c depends only on
# tanh chunk c (and halfA from chunk 0).
SCH = [(0, 17), (17, 33), (33, 49), (49, 65), (65, 81)]

_CACHE: dict = {}


def _build_nc():
    import concourse.bacc as bacc
    import concourse.tile as tile
    from concourse import mybir
    from contextlib import ExitStack

    nc = bacc.Bacc("TRN2", target_bir_lowering=False, debug=False)
    predsT = nc.dram_tensor("predsT", [P, R, KPP], mybir.dt.float8e4, kind="ExternalInput")
    pa = nc.dram_tensor("pa", [P, 4, KPP], mybir.dt.float32, kind="ExternalInput")
    aux = nc.dram_tensor("aux", [P, 4, KPP], mybir.dt.bfloat16, kind="ExternalInput")
    scores = nc.dram_tensor("scores", [P, C, KPP], mybir.dt.bfloat16, kind="ExternalOutput")
    boxes = nc.dram_tensor("boxes", [P, 4, KPP], mybir.dt.bfloat16, kind="ExternalOutput")

    fp32 = mybir.dt.float32
    bf16 = mybir.dt.bfloat16
    AF = mybir.ActivationFunctionType
    OP = mybir.AluOpType

    with tile.TileContext(nc) as tc, ExitStack() as ctx:
        pool = ctx.enter_context(tc.tile_pool(name="m", bufs=1))

        pt = pool.tile([P, R, KPP], mybir.dt.float8e4, tag="pt")
        pa_t = pool.tile([P, 4, KPP], fp32, tag="pa")
        aux_t = pool.tile([P, 4, KPP], bf16, tag="aux")

        # Input ring (sync HWDGE): preds chunk 0 first -- it heads the ACT
        # critical chain; box inputs next (needed by the early exp); then
        # the remaining preds chunks, FIFO ahead of when ACT needs them.
        r0, r1 = SCH[0]
        nc.sync.dma_start(out=pt[:, r0:r1, :], in_=predsT[:, r0:r1, :])
        nc.sync.dma_start(out=pa_t[:], in_=pa[:])
        nc.sync.dma_start(out=aux_t[:], in_=aux[:])
        for r0, r1 in SCH[1:]:
            nc.sync.dma_start(out=pt[:, r0:r1, :], in_=predsT[:, r0:r1, :])

        th = pool.tile([P, R, KPP], bf16, tag="th")
        wh_t = pool.tile([P, 2, KPP], fp32, tag="wh")

        # ACT: tanh chunk 0, then the (tiny) exp for boxes, then the rest of
        # the tanh ladder. One table set (exp_and_others) covers both.
        r0, r1 = SCH[0]
        nc.scalar.activation(th[:, r0:r1, :], pt[:, r0:r1, :], AF.Tanh, scale=0.5)
        nc.scalar.activation(wh_t[:], pa_t[:, 2:4, :], AF.Exp)
        for r0, r1 in SCH[1:]:
            nc.scalar.activation(th[:, r0:r1, :], pt[:, r0:r1, :], AF.Tanh, scale=0.5)

        halfA = pool.tile([P, KPP], bf16, tag="ha")
        sc_t = pool.tile([P, C, KPP], bf16, tag="sc")
        xy_s = pool.tile([P, 2, KPP], fp32, tag="xys")
        xy1 = pool.tile([P, 2, KPP], fp32, tag="xy1")
        wh_s = pool.tile([P, 2, KPP], fp32, tag="whs")
        box_t = pool.tile([P, 4, KPP], bf16, tag="box")

        # halfA = 0.25*(1+tanh(o/2)) = 0.5*sigma(obj)
        nc.vector.tensor_scalar(halfA[:], th[:, 0, :], 1.0, 0.25, OP.add, OP.mult)

        def score_chunk(c):
            r0, r1 = SCH[c]
            s0, s1 = r0 - 1 + (1 if c == 0 else 0), r1 - 1
            t0 = r0 + (1 if c == 0 else 0)
            nc.vector.scalar_tensor_tensor(
                sc_t[:, s0:s1, :],
                th[:, t0:r1, :],
                1.0,
                halfA[:].unsqueeze(1).broadcast_to([P, s1 - s0, KPP]),
                OP.add,
                OP.mult,
            )
            nc.scalar.dma_start(out=scores[:, s0:s1, :], in_=sc_t[:, s0:s1, :])

        score_chunk(0)

        # Box decode: x1 = px*s + bx, x2 = x1 + exp(pw)*s (y alike). The
        # /image_shape then *W,H of the reference cancels (H == W == 640).
        nc.vector.tensor_mul(xy_s[:], pa_t[:, 0:2, :], aux_t[:, 0:2, :])
        nc.vector.tensor_add(xy1[:], xy_s[:], aux_t[:, 2:4, :])
        nc.vector.tensor_copy(box_t[:, 0:2, :], xy1[:])
        nc.vector.tensor_mul(wh_s[:], wh_t[:], aux_t[:, 0:2, :])
        nc.vector.tensor_add(box_t[:, 2:4, :], xy1[:], wh_s[:])

        for c in range(1, len(SCH)):
            score_chunk(c)
        nc.scalar.dma_start(out=boxes[:], in_=box_t[:])

    nc.compile()
    return nc


def _host_consts():
    # Per-anchor stride s and grid offsets bx = gx*s, by = gy*s, padded to
    # N_PAD, replicated for the 2 images per core, as [P, 4, KPP] planes
    # (s, s, bx, by). All values are exact in bf16.
    s = np.ones(N_PAD, np.float32)
    bx = np.zeros(N_PAD, np.float32)
    by = np.zeros(N_PAD, np.float32)
    off = 0
    for g, st in ((80, 8.0), (40, 16.0), (20, 32.0)):
        n = g * g
        i = np.arange(n)
        s[off : off + n] = st
        bx[off : off + n] = (i % g) * st
        by[off : off + n] = (i // g) * st
        off += n
    pl = np.stack([s, s, bx, by], 0)                     # [4, N_PAD]
    pl = np.concatenate([pl] * B_PER_CORE, 1)            # [4, 2*N_PAD]
    aux = pl.reshape(4, P, KPP).transpose(1, 0, 2)       # [P, 4, KPP]
    return np.ascontiguousarray(aux.astype(ml_dtypes.bfloat16))


def _host_in_maps(pred0, pred1, pred2):
    aux = _CACHE["consts"]
    pred0 = np.asarray(pred0, np.float32).reshape(B, -1, F)
    pred1 = np.asarray(pred1, np.float32).reshape(B, -1, F)
    pred2 = np.asarray(pred2, np.float32).reshape(B, -1, F)
    in_maps = []
    for core in range(N_CORES):
        flat = np.zeros((B_PER_CORE * N_PAD, F), np.float32)
        for j in range(B_PER_CORE):
            b = core * B_PER_CORE + j
            flat[j * N_PAD : j * N_PAD + N_REAL] = np.concatenate(
                [pred0[b], pred1[b], pred2[b]], axis=0
            )
        a = flat.reshape(P, KPP, F)                      # [p, k, field]
        predsT = np.empty((P, R, KPP), np.float32)
        predsT[:, 0, :] = a[:, :, 4]
        predsT[:, 1:, :] = a[:, :, 5:].transpose(0, 2, 1)
        in_maps.append(
            {
                "predsT": predsT.astype(ml_dtypes.float8_e4m3fn),
                "pa": np.ascontiguousarray(a[:, :, 0:4].transpose(0, 2, 1)),
                "aux": aux,
            }
        )
    return in_maps


def kernel(images, pred0, pred1, pred2):
    from concourse.bass_utils import run_bass_kernel_spmd

    if "nc" not in _CACHE:
        _CACHE["consts"] = _host_consts()
        _CACHE["nc"] = _build_nc()
    nc = _CACHE["nc"]

    in_maps = _host_in_maps(pred0, pred1, pred2)
    res = run_bass_kernel_spmd(nc, in_maps, list(range(N_CORES)))

    final = np.empty((B, N_REAL * C, 6), np.float32)
    v = final.reshape(B, N_REAL, C, 6)
    v[..., 4] = np.arange(C, dtype=np.float32)[None, None, :]
    for core, r in enumerate(res.results):
        # [P, C, KPP] -> per-image [N_REAL, C]; [P, 4, KPP] -> [N_REAL, 4]
        sc = (
            r["scores"].astype(np.float32)
            .reshape(B_PER_CORE, P // B_PER_CORE, C, KPP)
            .transpose(0, 1, 3, 2)
            .reshape(B_PER_CORE, N_PAD, C)
        )
        bx = (
            r["boxes"].astype(np.float32)
            .reshape(B_PER_CORE, P // B_PER_CORE, 4, KPP)
            .transpose(0, 1, 3, 2)
            .reshape(B_PER_CORE, N_PAD, 4)
        )
        for j in range(B_PER_CORE):
            b = core * B_PER_CORE + j
            v[b, :, :, 0:4] = bx[j, :N_REAL, None, :]
            v[b, :, :, 5] = sc[j, :N_REAL, :]
    return final


# revision 3
# speedup vs baseline: 2.9007x; 1.1025x over previous
"""Trainium2 Bass kernel for nn_DecodePredictions (YOLO-style decode, B=16).

Pure data-parallel over batch (2 images per core x 8 cores).

The [B, N*C, 6] output is hugely redundant on the device side: per anchor,
the 4 box coords repeat across all 80 classes and lane 4 is the constant
class id. The device therefore emits only the per-anchor uniques --
4 box coords (bf16) and 80 class scores (bf16) -- ~1.5 MB/core instead of
the 16 MB/core the full layout costs; the host broadcasts them into the
full [B, N*C, 6] fp32 array while unsharding.

Scores: sigma on ACT (the only engine with activation LUTs), then one
tensor_tensor multiply per class chunk against broadcast sigma(obj) --
all-bf16 step-1 APs keep the DVE in its 2x packed mode (the fused
scalar_tensor_tensor alternative only has a 1x uop and measures 2x
slower). Box wh avoids the Exp table entirely -- exp(w) =
sigma(w)/(1-sigma(w)) via the DVE's hardware-divide reciprocal on the
tiny [P, 264] plane -- so the whole kernel uses ONE ACT table set and
pays a single ACT_TABLE_LOAD, which hides under the NEFF preamble.

Layouts are class-major [P, 81, KPP] (anchor innermost) so every ACT/DVE
op is step-1 contiguous and every DMA moves 128 contiguous per-partition
segments of 2-5 KB. Input DMAs are split across both HWDGE rings (sync +
scalar sequencers issue descriptors concurrently, ~600ns each); score
chunks stream out as their multiplies finish, boxes slot into the out
ring mid-stream, and the last class chunk is small to shorten the
drain tail.
"""

import ml_dtypes
import numpy as np

N_CORES = 8
B = 16
B_PER_CORE = B // N_CORES  # 2
C = 80
F = 85
N_REAL = 8400              # 80*80 + 40*40 + 20*20
N_PAD = 8448               # = 66 * 128
P = 128
KPP = B_PER_CORE * N_PAD // P  # 132 anchors per partition
R = C + 1                  # obj row + 80 class rows
# sigma-row chunks [r0, r1); chunk 0 carries the obj row. Score rows are
# the same ranges shifted down by one, so multiply chunk c depends only on
# sigma chunk c (plus sigma(obj) from chunk 0).
SCH = [(0, 17), (17, 35), (35, 53), (53, 71), (71, 81)]

_CACHE: dict = {}


def _build_nc():
    import concourse.bacc as bacc
    import concourse.tile as tile
    from concourse import mybir
    from contextlib import ExitStack

    nc = bacc.Bacc("TRN2", target_bir_lowering=False, debug=False)
    predsT = nc.dram_tensor("predsT", [P, R, KPP], mybir.dt.float8e4, kind="ExternalInput")
    pa = nc.dram_tensor("pa", [P, 4, KPP], mybir.dt.float32, kind="ExternalInput")
    aux = nc.dram_tensor("aux", [P, 4, KPP], mybir.dt.bfloat16, kind="ExternalInput")
    scores = nc.dram_tensor("scores", [P, C, KPP], mybir.dt.bfloat16, kind="ExternalOutput")
    boxes = nc.dram_tensor("boxes", [P, 4, KPP], mybir.dt.bfloat16, kind="ExternalOutput")

    fp32 = mybir.dt.float32
    bf16 = mybir.dt.bfloat16
    AF = mybir.ActivationFunctionType
    OP = mybir.AluOpType

    with tile.TileContext(nc) as tc, ExitStack() as ctx:
        pool = ctx.enter_context(tc.tile_pool(name="m", bufs=1))

        pt = pool.tile([P, R, KPP], mybir.dt.float8e4, tag="pt")
        pa_t = pool.tile([P, 4, KPP], fp32, tag="pa")
        aux_t = pool.tile([P, 4, KPP], bf16, tag="aux")

        # Input DMAs split across both HWDGE rings so descriptor issue
        # (~600ns each, serialized per sequencer) runs in parallel.
        # scalar ring: preds chunk 0 (heads the ACT critical chain).
        # sync ring:   pa, aux (box path), then the remaining preds rows.
        r0, r1 = SCH[0]
        nc.scalar.dma_start(out=pt[:, r0:r1, :], in_=predsT[:, r0:r1, :])
        nc.sync.dma_start(out=pa_t[:], in_=pa[:])
        nc.sync.dma_start(out=aux_t[:], in_=aux[:])
        nc.sync.dma_start(out=pt[:, 17:49, :], in_=predsT[:, 17:49, :])
        nc.sync.dma_start(out=pt[:, 49:81, :], in_=predsT[:, 49:81, :])

        sg = pool.tile([P, R, KPP], bf16, tag="sg")
        sp = pool.tile([P, 2, KPP], fp32, tag="sp")

        # ACT ladder: sigma chunk 0, the tiny box sigma, then the rest.
        r0, r1 = SCH[0]
        nc.scalar.activation(sg[:, r0:r1, :], pt[:, r0:r1, :], AF.Sigmoid)
        nc.scalar.activation(sp[:], pa_t[:, 2:4, :], AF.Sigmoid)
        for r0, r1 in SCH[1:]:
            nc.scalar.activation(sg[:, r0:r1, :], pt[:, r0:r1, :], AF.Sigmoid)

        sc_t = pool.tile([P, C, KPP], bf16, tag="sc")
        om_t = pool.tile([P, 2, KPP], fp32, tag="om")
        rc_t = pool.tile([P, 2, KPP], fp32, tag="rc")
        wh_t = pool.tile([P, 2, KPP], fp32, tag="wh")
        xy_s = pool.tile([P, 2, KPP], fp32, tag="xys")
        xy1 = pool.tile([P, 2, KPP], fp32, tag="xy1")
        wh_s = pool.tile([P, 2, KPP], fp32, tag="whs")
        box_t = pool.tile([P, 4, KPP], bf16, tag="box")

        def score_chunk(c):
            r0, r1 = SCH[c]
            s0, t0 = (0, 1) if c == 0 else (r0 - 1, r0)
            s1 = r1 - 1
            nc.vector.tensor_mul(
                sc_t[:, s0:s1, :],
                sg[:, t0:r1, :],
                sg[:, 0, :].unsqueeze(1).broadcast_to([P, s1 - s0, KPP]),
            )
            nc.scalar.dma_start(out=scores[:, s0:s1, :], in_=sc_t[:, s0:s1, :])

        score_chunk(0)
        score_chunk(1)

        # Box decode: x1 = px*s + bx, x2 = x1 + exp(pw)*s (y alike; the
        # /image_shape then *W,H of the reference cancels, H == W == 640).
        # exp(w) = sigma(w) / (1 - sigma(w)), reciprocal on DVE.
        nc.vector.tensor_scalar(om_t[:], sp[:], 1.0, -1.0, OP.subtract, OP.mult)
        nc.vector.reciprocal(rc_t[:], om_t[:])
        nc.vector.tensor_mul(wh_t[:], sp[:], rc_t[:])
        nc.vector.tensor_mul(xy_s[:], pa_t[:, 0:2, :], aux_t[:, 0:2, :])
        nc.vector.tensor_add(xy1[:], xy_s[:], aux_t[:, 2:4, :])
        nc.vector.tensor_copy(box_t[:, 0:2, :], xy1[:])
        nc.vector.tensor_mul(wh_s[:], wh_t[:], aux_t[:, 0:2, :])
        nc.vector.tensor_add(box_t[:, 2:4, :], xy1[:], wh_s[:])
        nc.scalar.dma_start(out=boxes[:], in_=box_t[:])

        for c in range(2, len(SCH)):
            score_chunk(c)

    nc.compile()
    return nc


def _host_consts():
    # Per-anchor stride s and grid offsets bx = gx*s, by = gy*s, padded to
    # N_PAD, replicated for the 2 images per core, as [P, 4, KPP] planes
    # (s, s, bx, by). All values are exact in bf16.
    s = np.ones(N_PAD, np.float32)
    bx = np.zeros(N_PAD, np.float32)
    by = np.zeros(N_PAD, np.float32)
    off = 0
    for g, st in ((80, 8.0), (40, 16.0), (20, 32.0)):
        n = g * g
        i = np.arange(n)
        s[off : off + n] = st
        bx[off : off + n] = (i % g) * st
        by[off : off + n] = (i // g) * st
        off += n
    pl = np.stack([s, s, bx, by], 0)                     # [4, N_PAD]
    pl = np.concatenate([pl] * B_PER_CORE, 1)            # [4, 2*N_PAD]
    aux = pl.reshape(4, P, KPP).transpose(1, 0, 2)       # [P, 4, KPP]
    return np.ascontiguousarray(aux.astype(ml_dtypes.bfloat16))


def _host_in_maps(pred0, pred1, pred2):
    aux = _CACHE["consts"]
    pred0 = np.asarray(pred0, np.float32).reshape(B, -1, F)
    pred1 = np.asarray(pred1, np.float32).reshape(B, -1, F)
    pred2 = np.asarray(pred2, np.float32).reshape(B, -1, F)
    in_maps = []
    for core in range(N_CORES):
        flat = np.zeros((B_PER_CORE * N_PAD, F), np.float32)
        for j in range(B_PER_CORE):
            b = core * B_PER_CORE + j
            flat[j * N_PAD : j * N_PAD + N_REAL] = np.concatenate(
                [pred0[b], pred1[b], pred2[b]], axis=0
            )
        a = flat.reshape(P, KPP, F)                      # [p, k, field]
        predsT = np.empty((P, R, KPP), np.float32)
        predsT[:, 0, :] = a[:, :, 4]
        predsT[:, 1:, :] = a[:, :, 5:].transpose(0, 2, 1)
        in_maps.append(
            {
                "predsT": predsT.astype(ml_dtypes.float8_e4m3fn),
                "pa": np.ascontiguousarray(a[:, :, 0:4].transpose(0, 2, 1)),
                "aux": aux,
            }
        )
    return in_maps


def kernel(images, pred0, pred1, pred2):
    from concourse.bass_utils import run_bass_kernel_spmd

    if "nc" not in _CACHE:
        _CACHE["consts"] = _host_consts()
        _CACHE["nc"] = _build_nc()
    nc = _CACHE["nc"]

    in_maps = _host_in_maps(pred0, pred1, pred2)
    res = run_bass_kernel_spmd(nc, in_maps, list(range(N_CORES)))

    final = np.empty((B, N_REAL * C, 6), np.float32)
    v = final.reshape(B, N_REAL, C, 6)
    v[..., 4] = np.arange(C, dtype=np.float32)[None, None, :]
    for core, r in enumerate(res.results):
        # [P, C, KPP] -> per-image [N_REAL, C]; [P, 4, KPP] -> [N_REAL, 4]
        sc = (
            r["scores"].astype(np.float32)
            .reshape(B_PER_CORE, P // B_PER_CORE, C, KPP)
            .transpose(0, 1, 3, 2)
            .reshape(B_PER_CORE, N_PAD, C)
        )
        bx = (
            r["boxes"].astype(np.float32)
            .reshape(B_PER_CORE, P // B_PER_CORE, 4, KPP)
            .transpose(0, 1, 3, 2)
            .reshape(B_PER_CORE, N_PAD, 4)
        )
        for j in range(B_PER_CORE):
            b = core * B_PER_CORE + j
            v[b, :, :, 0:4] = bx[j, :N_REAL, None, :]
            v[b, :, :, 5] = sc[j, :N_REAL, :]
    return final
